# revision 11
# baseline (speedup 1.0000x reference)
# kernel.py — Bahdanau additive-attention block on 8 Trainium2 NeuronCores.
#
# reference:
#   ws      = s @ W_a_w.T + W_a_b                      [B, DFF]
#   uh      = einsum('bte,fe->btf', h, U_a_w) + U_a_b  [B, T, DFF]
#   x       = tanh(ws[:, None, :] + uh)                [B, T, DFF]
#   energy  = x @ V_a_w[0] + V_a_b[0]                  [B, T]
#   context = einsum('bt,bte->be', energy, h)          [B, 2H]
#
# Sharding: data-parallel over batch B=32 across 8 cores (4 batches/core),
# weights replicated. No collectives needed.
#
# Per-core dataflow (h read from HBM exactly once):
#   - h tile [t=128, e] loaded natural, PE-transposed to hT [e=128, t]
#   - uh^T [f, t] = U^T (stationary) x hT (moving), accumulated over e-chunks
#   - x^T = tanh(uh^T + ws^T_b) via ScalarE with per-partition bias
#   - energy^T [t, 1] = x^T-chunk (stationary) x V (moving), accum over f
#   - context [1, e] += energy^T (stationary) x h-natural (moving), over t
#
# Matmul operands use float32r (fp32 rounded for the PE fast path, 4x the
# plain-fp32 matmul rate); accumulation is fp32 in PSUM.
import numpy as np

B, T, H, DFF = 32, 4096, 512, 512
E = 2 * H            # 1024
NCORES = 8
BLOC = B // NCORES   # 4
TT = 512             # t-rows per pipeline tile
NTT = T // TT        # 8 tiles per batch
P = 128

# matmul dtype: "f32r" (4x faster PE, fp32 w/ rounded mantissa) or "f32"
MM_DTYPE = "f32r"

_cache = {}


def _build(mm_dtype):
    from concourse import bacc, tile, mybir, masks

    f32 = mybir.dt.float32
    mmdt = mybir.dt.float32r if mm_dtype == "f32r" else f32

    nc = bacc.Bacc("TRN2", target_bir_lowering=False, debug=False,
                   num_devices=NCORES)

    h_ext = nc.dram_tensor("h_loc", [BLOC, T, E], f32, kind="ExternalInput")
    sT_ext = nc.dram_tensor("sT4", [P, 4 * BLOC], f32, kind="ExternalInput")
    WT_ext = nc.dram_tensor("WT4", [P, 4 * DFF], f32, kind="ExternalInput")
    UT_ext = nc.dram_tensor("UT8", [P, (E // P) * DFF], f32, kind="ExternalInput")
    bf_ext = nc.dram_tensor("bfold", [P, DFF // P], f32, kind="ExternalInput")
    VT_ext = nc.dram_tensor("VT4", [P, DFF // P], f32, kind="ExternalInput")
    vb_ext = nc.dram_tensor("vb128", [P, 1], f32, kind="ExternalInput")
    out_ext = nc.dram_tensor("ctx_loc", [BLOC, E], f32, kind="ExternalOutput")

    FCH = DFF // P   # 4 f-chunks
    ECH = E // P     # 8 e-chunks
    HCH = H // P     # 4 h-chunks (for ws matmul)
    NSUB = TT // P   # 4 t-sub-chunks per tile

    with tile.TileContext(nc) as tc:
        with (
            tc.tile_pool(name="const", bufs=1) as cpool,
            tc.tile_pool(name="hnat", bufs=3) as hpool,
            tc.tile_pool(name="ht", bufs=2) as htpool,
            tc.tile_pool(name="xt", bufs=8) as xpool,
            tc.tile_pool(name="et", bufs=4) as etpool,
            tc.tile_pool(name="osb", bufs=2) as opool,
            tc.tile_pool(name="ps_tr", bufs=2, space="PSUM") as trpool,
            tc.tile_pool(name="ps_uh", bufs=3, space="PSUM") as uhpool,
            tc.tile_pool(name="ps_e", bufs=1, space="PSUM") as epool,
            tc.tile_pool(name="ps_ctx", bufs=1, space="PSUM") as ctxpool,
        ):
            # ---- constants / weights (matmul operands in mmdt via cast-DMA) ----
            ident_f32 = cpool.tile([P, P], f32)
            masks.make_identity(nc, ident_f32[:])
            ident = cpool.tile([P, P], mmdt)
            nc.vector.tensor_copy(ident[:], ident_f32[:])

            sT4 = cpool.tile([P, 4 * BLOC], f32)
            WT4 = cpool.tile([P, 4 * DFF], f32)
            UT8 = cpool.tile([P, ECH * DFF], mmdt)
            VT4 = cpool.tile([P, FCH], f32)
            bfold = cpool.tile([P, FCH], f32)
            vb128 = cpool.tile([P, 1], f32)
            nc.sync.dma_start(sT4[:], sT_ext[:])
            nc.sync.dma_start(WT4[:], WT_ext[:])
            nc.gpsimd.dma_start(UT8[:], UT_ext[:])
            nc.sync.dma_start(VT4[:], VT_ext[:])
            nc.sync.dma_start(bfold[:], bf_ext[:])
            nc.sync.dma_start(vb128[:], vb_ext[:])

            # ---- ws^T = W @ s^T  -> [f, b] with folded bias ----
            # wsb[:, fc*BLOC + b] = (s @ W.T + W_b + U_b)^T [f-chunk fc]
            wsb = cpool.tile([P, FCH * BLOC], f32)
            for fc in range(FCH):
                ps_ws = epool.tile([P, BLOC], f32, tag="ps_e")
                for hc in range(HCH):
                    nc.tensor.matmul(
                        ps_ws[:],
                        WT4[:, hc * DFF + fc * P: hc * DFF + (fc + 1) * P],
                        sT4[:, hc * BLOC:(hc + 1) * BLOC],
                        start=(hc == 0), stop=(hc == HCH - 1),
                    )
                nc.vector.tensor_scalar(
                    out=wsb[:, fc * BLOC:(fc + 1) * BLOC],
                    in0=ps_ws[:], scalar1=bfold[:, fc:fc + 1], scalar2=None,
                    op0=mybir.AluOpType.add,
                )

            # ---- main loop ----
            for b in range(BLOC):
                ctx_ps = ctxpool.tile([1, E], f32)
                for tt in range(NTT):
                    t0 = tt * TT
                    # load h natural (rounded to mmdt): [p=t_in, (i, e)]
                    h_nat = hpool.tile([P, NSUB * E], mmdt)
                    nc.gpsimd.dma_start(
                        h_nat[:].rearrange("p (i e) -> p i e", e=E),
                        h_ext[b, t0:t0 + TT, :].rearrange(
                            "(i p) e -> p i e", p=P),
                    )
                    # transpose to hT: [p=e_in_chunk, (c, t)]
                    hT = htpool.tile([P, ECH * TT], mmdt)
                    for c in range(ECH):
                        ps_tr = trpool.tile([P, TT], mmdt)
                        for i in range(NSUB):
                            nc.tensor.transpose(
                                ps_tr[:, i * P:(i + 1) * P],
                                h_nat[:, i * E + c * P: i * E + (c + 1) * P],
                                ident[:],
                            )
                        nc.vector.tensor_copy(
                            hT[:, c * TT:(c + 1) * TT], ps_tr[:])

                    # uh^T per f-chunk + tanh
                    xts = []
                    for fc in range(FCH):
                        ps_uh = uhpool.tile([P, TT], f32)
                        for c in range(ECH):
                            nc.tensor.matmul(
                                ps_uh[:],
                                UT8[:, c * DFF + fc * P: c * DFF + (fc + 1) * P],
                                hT[:, c * TT:(c + 1) * TT],
                                start=(c == 0), stop=(c == ECH - 1),
                            )
                        xT = xpool.tile([P, TT], f32, tag="xt")
                        nc.scalar.activation(
                            xT[:], ps_uh[:],
                            mybir.ActivationFunctionType.Tanh,
                            bias=wsb[:, fc * BLOC + b: fc * BLOC + b + 1],
                        )
                        xts.append(xT)

                    # energy^T [t, 1] per sub-chunk, all in one psum tile
                    ps_e = epool.tile([P, NSUB], f32, tag="ps_e")
                    for i in range(NSUB):
                        for fc in range(FCH):
                            nc.tensor.matmul(
                                ps_e[:, i:i + 1],
                                xts[fc][:, i * P:(i + 1) * P],
                                VT4[:, fc:fc + 1],
                                start=(fc == 0), stop=(fc == FCH - 1),
                            )
                    eT = etpool.tile([P, NSUB], mmdt)
                    nc.vector.tensor_scalar(
                        out=eT[:], in0=ps_e[:], scalar1=vb128[:, 0:1],
                        scalar2=None, op0=mybir.AluOpType.add,
                    )

                    # context += energy^T x h_natural
                    for i in range(NSUB):
                        for half in range(2):
                            nc.tensor.matmul(
                                ctx_ps[:, half * 512:(half + 1) * 512],
                                eT[:, i:i + 1],
                                h_nat[:, i * E + half * 512:
                                      i * E + half * 512 + 512],
                                start=(tt == 0 and i == 0),
                                stop=(tt == NTT - 1 and i == NSUB - 1),
                            )
                # write out context for this b
                ctx_sb = opool.tile([1, E], f32)
                nc.vector.tensor_copy(ctx_sb[:, 0:512], ctx_ps[:, 0:512])
                nc.vector.tensor_copy(ctx_sb[:, 512:1024], ctx_ps[:, 512:1024])
                nc.sync.dma_start(out_ext[b:b + 1, :], ctx_sb[:])

    nc.compile()
    return nc


def prepare(s, h, W_a_w, W_a_b, U_a_w, U_a_b, V_a_w, V_a_b):
    """Build (cached) the Bass module and the per-core input maps."""
    s = np.asarray(s, dtype=np.float32)
    h = np.ascontiguousarray(np.asarray(h, dtype=np.float32))
    W_a_w = np.asarray(W_a_w, dtype=np.float32)
    W_a_b = np.asarray(W_a_b, dtype=np.float32)
    U_a_w = np.asarray(U_a_w, dtype=np.float32)
    U_a_b = np.asarray(U_a_b, dtype=np.float32)
    V_a_w = np.asarray(V_a_w, dtype=np.float32)
    V_a_b = np.asarray(V_a_b, dtype=np.float32)

    if "nc" not in _cache:
        _cache["nc"] = _build(MM_DTYPE)
    nc = _cache["nc"]

    # host-side layout prep (weights are tiny; h is sliced per-core, no copy)
    WT4 = np.ascontiguousarray(
        W_a_w.T.reshape(4, P, DFF).transpose(1, 0, 2).reshape(P, 4 * DFF))
    UT8 = np.ascontiguousarray(
        U_a_w.T.reshape(E // P, P, DFF).transpose(1, 0, 2).reshape(P, -1))
    bfold = np.ascontiguousarray((W_a_b + U_a_b).reshape(DFF // P, P).T)
    VT4 = np.ascontiguousarray(V_a_w[0].reshape(DFF // P, P).T)
    vb128 = np.ascontiguousarray(
        np.broadcast_to(V_a_b.reshape(1, 1), (P, 1)).astype(np.float32))

    in_maps = []
    for core in range(NCORES):
        b0 = core * BLOC
        s_loc = s[b0:b0 + BLOC]                       # [4, 512]
        sT4 = np.ascontiguousarray(
            s_loc.T.reshape(4, P, BLOC).transpose(1, 0, 2).reshape(P, -1))
        in_maps.append({
            "h_loc": h[b0:b0 + BLOC],
            "sT4": sT4,
            "WT4": WT4,
            "UT8": UT8,
            "bfold": bfold,
            "VT4": VT4,
            "vb128": vb128,
        })

    return nc, in_maps


def kernel(s, h, W_a_w, W_a_b, U_a_w, U_a_b, V_a_w, V_a_b):
    from concourse.bass_utils import run_bass_kernel_spmd

    nc, in_maps = prepare(s, h, W_a_w, W_a_b, U_a_w, U_a_b, V_a_w, V_a_b)
    res = run_bass_kernel_spmd(nc, in_maps, list(range(NCORES)))
    out = np.concatenate([res.results[i]["ctx_loc"] for i in range(NCORES)],
                         axis=0)
    return out.astype(np.float32)


# revision 13
# speedup vs baseline: 1.1973x; 1.1973x over previous
# kernel.py — Bahdanau additive-attention block on 8 Trainium2 NeuronCores.
#
# reference:
#   ws      = s @ W_a_w.T + W_a_b                      [B, DFF]
#   uh      = einsum('bte,fe->btf', h, U_a_w) + U_a_b  [B, T, DFF]
#   x       = tanh(ws[:, None, :] + uh)                [B, T, DFF]
#   energy  = x @ V_a_w[0] + V_a_b[0]                  [B, T]
#   context = einsum('bt,bte->be', energy, h)          [B, 2H]
#
# Sharding: data-parallel over batch B=32 across 8 cores (4 batches/core),
# weights replicated. No collectives needed.
#
# Per-core dataflow (h read from HBM exactly once):
#   - h tile [t=128, e] loaded natural, PE-transposed to hT [e=128, t]
#   - uh^T [f, t] = U^T (stationary) x hT (moving), accumulated over e-chunks
#   - x^T = tanh(uh^T + ws^T_b) via ScalarE with per-partition bias
#   - energy^T [t, 1] = x^T-chunk (stationary) x V (moving), accum over f
#   - context [1, e] += energy^T (stationary) x h-natural (moving), over t
#
# Matmul operands use float32r (fp32 rounded for the PE fast path, 4x the
# plain-fp32 matmul rate); accumulation is fp32 in PSUM.
import numpy as np

B, T, H, DFF = 32, 4096, 512, 512
E = 2 * H            # 1024
NCORES = 8
BLOC = B // NCORES   # 4
TT = 512             # t-rows per pipeline tile
NTT = T // TT        # 8 tiles per batch
P = 128

# matmul dtype: "f32r" (4x faster PE, fp32 w/ rounded mantissa) or "f32"
MM_DTYPE = "f32r"

_cache = {}


def _build(mm_dtype):
    from concourse import bacc, tile, mybir, masks

    f32 = mybir.dt.float32
    mmdt = mybir.dt.float32r if mm_dtype == "f32r" else f32

    nc = bacc.Bacc("TRN2", target_bir_lowering=False, debug=False,
                   num_devices=NCORES)

    h_ext = nc.dram_tensor("h_loc", [BLOC, T, E], f32, kind="ExternalInput")
    sT_ext = nc.dram_tensor("sT4", [P, 4 * BLOC], f32, kind="ExternalInput")
    WT_ext = nc.dram_tensor("WT4", [P, 4 * DFF], f32, kind="ExternalInput")
    UT_ext = nc.dram_tensor("UT8", [P, (E // P) * DFF], f32, kind="ExternalInput")
    bf_ext = nc.dram_tensor("bfold", [P, DFF // P], f32, kind="ExternalInput")
    VT_ext = nc.dram_tensor("VT4", [P, DFF // P], f32, kind="ExternalInput")
    vb_ext = nc.dram_tensor("vb128", [P, 1], f32, kind="ExternalInput")
    out_ext = nc.dram_tensor("ctx_loc", [BLOC, E], f32, kind="ExternalOutput")

    FCH = DFF // P   # 4 f-chunks
    ECH = E // P     # 8 e-chunks
    HCH = H // P     # 4 h-chunks (for ws matmul)
    NSUB = TT // P   # 4 t-sub-chunks per tile

    with tile.TileContext(nc) as tc:
        with (
            tc.tile_pool(name="const", bufs=1) as cpool,
            tc.tile_pool(name="hnat", bufs=3) as hpool,
            tc.tile_pool(name="ht", bufs=2) as htpool,
            tc.tile_pool(name="xt", bufs=8) as xpool,
            tc.tile_pool(name="et", bufs=4) as etpool,
            tc.tile_pool(name="osb", bufs=2) as opool,
            tc.tile_pool(name="ps_tr", bufs=2, space="PSUM") as trpool,
            tc.tile_pool(name="ps_uh", bufs=2, space="PSUM") as uhpool,
            tc.tile_pool(name="ps_e", bufs=2, space="PSUM") as epool,
            tc.tile_pool(name="ps_ctx", bufs=1, space="PSUM") as ctxpool,
        ):
            # ---- constants / weights (matmul operands in mmdt via cast-DMA) ----
            ident_f32 = cpool.tile([P, P], f32)
            masks.make_identity(nc, ident_f32[:])
            ident = cpool.tile([P, P], mmdt)
            nc.vector.tensor_copy(ident[:], ident_f32[:])

            sT4 = cpool.tile([P, 4 * BLOC], f32)
            WT4 = cpool.tile([P, 4 * DFF], f32)
            UT8 = cpool.tile([P, ECH * DFF], mmdt)
            VT4 = cpool.tile([P, FCH], f32)
            bfold = cpool.tile([P, FCH], f32)
            vb128 = cpool.tile([P, 1], f32)
            nc.sync.dma_start(sT4[:], sT_ext[:])
            nc.sync.dma_start(WT4[:], WT_ext[:])
            nc.gpsimd.dma_start(UT8[:], UT_ext[:])
            nc.sync.dma_start(VT4[:], VT_ext[:])
            nc.sync.dma_start(bfold[:], bf_ext[:])
            nc.sync.dma_start(vb128[:], vb_ext[:])

            # ---- ws^T = W @ s^T  -> [f, b] with folded bias ----
            # wsb[:, fc*BLOC + b] = (s @ W.T + W_b + U_b)^T [f-chunk fc]
            wsb = cpool.tile([P, FCH * BLOC], f32)
            for fc in range(FCH):
                ps_ws = epool.tile([P, BLOC], f32, tag="ps_e")
                for hc in range(HCH):
                    nc.tensor.matmul(
                        ps_ws[:],
                        WT4[:, hc * DFF + fc * P: hc * DFF + (fc + 1) * P],
                        sT4[:, hc * BLOC:(hc + 1) * BLOC],
                        start=(hc == 0), stop=(hc == HCH - 1),
                    )
                nc.vector.tensor_scalar(
                    out=wsb[:, fc * BLOC:(fc + 1) * BLOC],
                    in0=ps_ws[:], scalar1=bfold[:, fc:fc + 1], scalar2=None,
                    op0=mybir.AluOpType.add,
                )

            # ---- main loop ----
            for b in range(BLOC):
                ctx_ps = ctxpool.tile([1, E], f32)
                for tt in range(NTT):
                    t0 = tt * TT
                    # load h natural (rounded to mmdt): [p=t_in, (i, e)]
                    h_nat = hpool.tile([P, NSUB * E], mmdt)
                    nc.gpsimd.dma_start(
                        h_nat[:].rearrange("p (i e) -> p i e", e=E),
                        h_ext[b, t0:t0 + TT, :].rearrange(
                            "(i p) e -> p i e", p=P),
                    )
                    # transpose to hT: [p=e_in_chunk, (c, t)]
                    hT = htpool.tile([P, ECH * TT], mmdt)
                    for c in range(ECH):
                        ps_tr = trpool.tile([P, TT], mmdt)
                        for i in range(NSUB):
                            nc.tensor.transpose(
                                ps_tr[:, i * P:(i + 1) * P],
                                h_nat[:, i * E + c * P: i * E + (c + 1) * P],
                                ident[:],
                            )
                        if c % 2 == 0:
                            nc.vector.tensor_copy(
                                hT[:, c * TT:(c + 1) * TT], ps_tr[:])
                        else:
                            nc.scalar.copy(
                                hT[:, c * TT:(c + 1) * TT], ps_tr[:])

                    # uh^T per f-chunk + tanh
                    xts = []
                    for fc in range(FCH):
                        ps_uh = uhpool.tile([P, TT], f32)
                        for c in range(ECH):
                            nc.tensor.matmul(
                                ps_uh[:],
                                UT8[:, c * DFF + fc * P: c * DFF + (fc + 1) * P],
                                hT[:, c * TT:(c + 1) * TT],
                                start=(c == 0), stop=(c == ECH - 1),
                            )
                        xT = xpool.tile([P, TT], f32, tag="xt")
                        nc.scalar.activation(
                            xT[:], ps_uh[:],
                            mybir.ActivationFunctionType.Tanh,
                            bias=wsb[:, fc * BLOC + b: fc * BLOC + b + 1],
                        )
                        xts.append(xT)

                    # energy^T [t, 1] per sub-chunk, all in one psum tile
                    ps_e = epool.tile([P, NSUB], f32, tag="ps_e")
                    for i in range(NSUB):
                        for fc in range(FCH):
                            nc.tensor.matmul(
                                ps_e[:, i:i + 1],
                                xts[fc][:, i * P:(i + 1) * P],
                                VT4[:, fc:fc + 1],
                                start=(fc == 0), stop=(fc == FCH - 1),
                            )
                    eT = etpool.tile([P, NSUB], mmdt)
                    nc.vector.tensor_scalar(
                        out=eT[:], in0=ps_e[:], scalar1=vb128[:, 0:1],
                        scalar2=None, op0=mybir.AluOpType.add,
                    )

                    # context += energy^T x h_natural
                    for i in range(NSUB):
                        for half in range(2):
                            nc.tensor.matmul(
                                ctx_ps[:, half * 512:(half + 1) * 512],
                                eT[:, i:i + 1],
                                h_nat[:, i * E + half * 512:
                                      i * E + half * 512 + 512],
                                start=(tt == 0 and i == 0),
                                stop=(tt == NTT - 1 and i == NSUB - 1),
                            )
                # write out context for this b
                ctx_sb = opool.tile([1, E], f32)
                nc.vector.tensor_copy(ctx_sb[:, 0:512], ctx_ps[:, 0:512])
                nc.vector.tensor_copy(ctx_sb[:, 512:1024], ctx_ps[:, 512:1024])
                nc.sync.dma_start(out_ext[b:b + 1, :], ctx_sb[:])

    nc.compile()
    return nc


def prepare(s, h, W_a_w, W_a_b, U_a_w, U_a_b, V_a_w, V_a_b):
    """Build (cached) the Bass module and the per-core input maps."""
    s = np.asarray(s, dtype=np.float32)
    h = np.ascontiguousarray(np.asarray(h, dtype=np.float32))
    W_a_w = np.asarray(W_a_w, dtype=np.float32)
    W_a_b = np.asarray(W_a_b, dtype=np.float32)
    U_a_w = np.asarray(U_a_w, dtype=np.float32)
    U_a_b = np.asarray(U_a_b, dtype=np.float32)
    V_a_w = np.asarray(V_a_w, dtype=np.float32)
    V_a_b = np.asarray(V_a_b, dtype=np.float32)

    if "nc" not in _cache:
        _cache["nc"] = _build(MM_DTYPE)
    nc = _cache["nc"]

    # host-side layout prep (weights are tiny; h is sliced per-core, no copy)
    WT4 = np.ascontiguousarray(
        W_a_w.T.reshape(4, P, DFF).transpose(1, 0, 2).reshape(P, 4 * DFF))
    UT8 = np.ascontiguousarray(
        U_a_w.T.reshape(E // P, P, DFF).transpose(1, 0, 2).reshape(P, -1))
    bfold = np.ascontiguousarray((W_a_b + U_a_b).reshape(DFF // P, P).T)
    VT4 = np.ascontiguousarray(V_a_w[0].reshape(DFF // P, P).T)
    vb128 = np.ascontiguousarray(
        np.broadcast_to(V_a_b.reshape(1, 1), (P, 1)).astype(np.float32))

    in_maps = []
    for core in range(NCORES):
        b0 = core * BLOC
        s_loc = s[b0:b0 + BLOC]                       # [4, 512]
        sT4 = np.ascontiguousarray(
            s_loc.T.reshape(4, P, BLOC).transpose(1, 0, 2).reshape(P, -1))
        in_maps.append({
            "h_loc": h[b0:b0 + BLOC],
            "sT4": sT4,
            "WT4": WT4,
            "UT8": UT8,
            "bfold": bfold,
            "VT4": VT4,
            "vb128": vb128,
        })

    return nc, in_maps


def _kernel_impl(s, h, W_a_w, W_a_b, U_a_w, U_a_b, V_a_w, V_a_b):
    from concourse.bass_utils import run_bass_kernel_spmd

    nc, in_maps = prepare(s, h, W_a_w, W_a_b, U_a_w, U_a_b, V_a_w, V_a_b)
    res = run_bass_kernel_spmd(nc, in_maps, list(range(NCORES)))
    out = np.concatenate([res.results[i]["ctx_loc"] for i in range(NCORES)],
                         axis=0)
    return out.astype(np.float32)


def _kernel_subprocess(**inputs):
    """Re-run in a fresh interpreter (fresh PJRT/axon client). The device
    occasionally reports NRT_EXEC_UNIT_UNRECOVERABLE transiently; a new
    process recovers it."""
    import os
    import subprocess
    import sys
    import tempfile

    mydir = os.path.dirname(os.path.abspath(__file__))
    with tempfile.TemporaryDirectory() as td:
        np.savez(os.path.join(td, "in.npz"), **inputs)
        code = (
            "import sys, numpy as np\n"
            f"sys.path.insert(0, {mydir!r})\n"
            "import kernel\n"
            f"d = np.load({os.path.join(td, 'in.npz')!r})\n"
            "out = kernel._kernel_impl(**{k: d[k] for k in d.files})\n"
            f"np.save({os.path.join(td, 'out.npy')!r}, out)\n"
        )
        subprocess.run([sys.executable, "-c", code], check=True, timeout=1800)
        return np.load(os.path.join(td, "out.npy"))


def kernel(s, h, W_a_w, W_a_b, U_a_w, U_a_b, V_a_w, V_a_b):
    inputs = dict(s=s, h=h, W_a_w=W_a_w, W_a_b=W_a_b, U_a_w=U_a_w,
                  U_a_b=U_a_b, V_a_w=V_a_w, V_a_b=V_a_b)
    try:
        return _kernel_impl(**inputs)
    except Exception as e:  # transient device-unrecoverable flake: retry
        msg = str(e)
        if not ("UNRECOVERABLE" in msg or "UNAVAILABLE" in msg
                or "PassThrough" in msg):
            raise
    last = None
    for _ in range(2):
        try:
            return _kernel_subprocess(**inputs)
        except Exception as e:
            last = e
    raise last
